# revision 1
# baseline (speedup 1.0000x reference)
"""Trainium2 Bass kernel for nn_Decoder: 11-step greedy LSTM decoder.

B=16, H=1024, V=32000, T=11 on 8 NeuronCores.
Sharding: tensor-parallel over vocab for the fc/logits matmul (each core holds
a [1024,4000] transposed shard of fc_W resident in SBUF), tensor-parallel over
hidden dim for the LSTM gates (each core computes h for its 128 hidden dims),
with a per-step AllGather of h^T chunks and an AllGather of per-core argmax
candidates (greedy feedback).  h0/c0 are never updated (reference semantics),
so h0@W_hh.T + b_ih + b_hh is precomputed once.

The fc matmul uses a 3-term bf16 split (W = Whi+Wlo, h = hhi+hlo;
h@W ~= hhi@Whi + hhi@Wlo + hlo@Whi) giving ~2^-17 relative precision at
bf16 streaming rate; verified to reproduce the fp32 greedy token path.
The gates matmul uses a 2-term split (x single bf16, W = Whi+Wlo): the
dropped x-residual term contributes ~2e-5 logit error vs a ~6e-3 minimum
top-2 gap, so the greedy path is unchanged (measured rel err 8.5e-06).
The exact 2x from the tanh-form sigmoid is folded into pre-scaled fc
weights (0.5*fc_W), which is exact.
"""

import numpy as np
import ml_dtypes

import concourse.bass as bass
import concourse.bacc as bacc
import concourse.tile as tile
import concourse.mybir as mybir
from concourse import bass_utils

B = 16
H = 1024
V = 32000
T = 11
NC = 8
SOS = 1
Vc = V // NC          # 4000 vocab rows per core
KT = H // 128         # 8 contraction tiles
NCHUNK = 8            # fc free-dim chunks
CW = Vc // NCHUNK     # 500 cols per chunk
GSL = 4 * 128         # 512 gate rows per core
NWARM1 = 12           # PE-warming dummies during h-AllGather
NWARM2 = 14           # PE-warming dummies during candidate-AllGather

F32 = mybir.dt.float32
BF16 = mybir.dt.bfloat16
U32 = mybir.dt.uint32
U8 = mybir.dt.uint8
AX = mybir.AxisListType
ALU = mybir.AluOpType
ACTF = mybir.ActivationFunctionType
BIG = 1.0e9

_CACHE: dict = {}


def _build(reps=1, startup_in_rep=True, coll=True, warm=False):
    nc = bacc.Bacc("TRN2", target_bir_lowering=False, debug=False, num_devices=NC)

    emb_d = nc.dram_tensor("emb", [V, H], F32, kind="ExternalInput")
    fcwh_d = nc.dram_tensor("fcwh", [KT, 128, Vc], BF16, kind="ExternalInput")
    fcwl_d = nc.dram_tensor("fcwl", [KT, 128, Vc], BF16, kind="ExternalInput")
    wihh_d = nc.dram_tensor("wihh", [KT, 128, GSL], BF16, kind="ExternalInput")
    wihl_d = nc.dram_tensor("wihl", [KT, 128, GSL], BF16, kind="ExternalInput")
    whht_d = nc.dram_tensor("whht", [KT, 128, GSL], F32, kind="ExternalInput")
    h0t_d = nc.dram_tensor("h0t", [KT, 128, B], F32, kind="ExternalInput")
    c0h_d = nc.dram_tensor("c0h", [B, 128], F32, kind="ExternalInput")
    bsum_d = nc.dram_tensor("bsum", [1, GSL], F32, kind="ExternalInput")
    fcbr_d = nc.dram_tensor("fcbr", [B, Vc], F32, kind="ExternalInput")
    id16_d = nc.dram_tensor("id16", [B, B], F32, kind="ExternalInput")
    cbase_d = nc.dram_tensor("cbase", [B, NCHUNK], F32, kind="ExternalInput")

    out_d = nc.dram_tensor("out", [B, T, Vc], F32, kind="ExternalOutput")
    tokdbg_d = nc.dram_tensor("tokdbg", [B, T + 1], U32, kind="ExternalOutput")
    wdbg_d = nc.dram_tensor("wdbg", [B, CW], F32, kind="ExternalOutput")

    with tile.TileContext(nc) as tc:
        with (
            tc.tile_pool(name="persist", bufs=1) as pp,
            tc.tile_pool(name="work", bufs=2) as wp,
            tc.tile_pool(name="small", bufs=1) as sp,
            tc.tile_pool(name="stream", bufs=1) as stp,
            tc.tile_pool(name="psum", bufs=1, space="PSUM") as psp,
            tc.tile_pool(name="psfc", bufs=3, space="PSUM") as psfc,
            tc.tile_pool(name="dram", bufs=2, space="DRAM") as dp,
        ):
            # ---------------- persistent tiles ----------------
            fcwh_sb = pp.tile([128, KT * Vc], BF16, tag="fcwh")
            fcwl_sb = pp.tile([128, KT * Vc], BF16, tag="fcwl")
            wihh_sb = pp.tile([128, KT * GSL], BF16, tag="wihh")
            wihl_sb = pp.tile([128, KT * GSL], BF16, tag="wihl")
            fcbr_sb = pp.tile([B, Vc], F32, tag="fcbr")
            bsum_sb = pp.tile([1, GSL], F32, tag="bsum")
            const_sb = pp.tile([B, GSL], F32, tag="const")
            c0h_sb = pp.tile([B, 128], F32, tag="c0h")
            id16_sb = pp.tile([B, B], F32, tag="id16")
            cbase_sb = pp.tile([B, NCHUNK], F32, tag="cbase")
            ones1_sb = pp.tile([1, B], F32, tag="ones1")
            big8_sb = pp.tile([B, NCHUNK], F32, tag="big8")
            hT2_sb = pp.tile([128, KT * 3 * B], BF16, tag="hT2")
            gmax_all = pp.tile([B, T], F32, tag="gmax")
            sume_all = pp.tile([B, T], F32, tag="sume")
            tok_all = pp.tile([B, T + 1], U32, tag="tok")
            ldram = dp.tile([B, T * Vc], F32, tag="ldram")

            ps_warm = psp.tile([B, CW], F32, tag="warm")

            for _rep in range(reps):
                if _rep == 0 or startup_in_rep:
                    # ---------------- startup ----------------
                    for k in range(KT):
                        nc.sync.dma_start(fcwh_sb[:, k * Vc:(k + 1) * Vc],
                                          fcwh_d[k, :, :])
                        nc.sync.dma_start(fcwl_sb[:, k * Vc:(k + 1) * Vc],
                                          fcwl_d[k, :, :])
                        nc.sync.dma_start(wihh_sb[:, k * GSL:(k + 1) * GSL],
                                          wihh_d[k, :, :])
                        nc.sync.dma_start(wihl_sb[:, k * GSL:(k + 1) * GSL],
                                          wihl_d[k, :, :])
                    nc.sync.dma_start(fcbr_sb[:], fcbr_d[:, :])
                    nc.sync.dma_start(bsum_sb[:], bsum_d[:, :])
                    nc.sync.dma_start(c0h_sb[:], c0h_d[:, :])
                    nc.sync.dma_start(id16_sb[:], id16_d[:, :])
                    nc.sync.dma_start(cbase_sb[:], cbase_d[:, :])
                    nc.vector.memset(ones1_sb[:], 1.0)
                    nc.vector.memset(big8_sb[:], BIG)
                    nc.vector.memset(hT2_sb[:], 0)

                    # const = h0 @ W_hh_slice.T + (b_ih+b_hh) slice  [B, 512]
                    ps_c = psp.tile([B, GSL], F32, tag="psc")
                    for k in range(KT):
                        h0tile = stp.tile([128, B], F32, tag="h0tmp")
                        nc.sync.dma_start(h0tile[:], h0t_d[k, :, :])
                        for hv in range(2):
                            hs = slice(hv * (GSL // 2), (hv + 1) * (GSL // 2))
                            whtile = stp.tile([128, GSL // 2], F32, tag="whtmp")
                            nc.sync.dma_start(whtile[:], whht_d[k, :, hs])
                            # start=True clears has_written for the WHOLE bank,
                            # so only the very first matmul may set it
                            nc.tensor.matmul(ps_c[:, hs], h0tile[:], whtile[:],
                                             start=(k == 0 and hv == 0),
                                             stop=False)
                    nc.tensor.matmul(ps_c[:], ones1_sb[:], bsum_sb[:],
                                     start=False, stop=True)
                    nc.scalar.copy(const_sb[:], ps_c[:])

                nc.vector.memset(tok_all[:], 0)
                nc.vector.memset(tok_all[:, 0:1], SOS)

                # ---------------- decode loop ----------------
                for t in range(T):
                    # gather x = emb[tok]  -> [B, H]
                    x_sb = sp.tile([B, H], F32, tag="x")
                    nc.gpsimd.indirect_dma_start(
                        out=x_sb[:],
                        out_offset=None,
                        in_=emb_d[:, :],
                        in_offset=bass.IndirectOffsetOnAxis(
                            ap=tok_all[:, t:t + 1], axis=0),
                    )

                    # transpose x -> [128, KT*B], relu, bf16 split
                    ps_xt = psp.tile([128, KT * B], F32, tag="psxt")
                    for k in range(KT):
                        nc.tensor.transpose(
                            ps_xt[:, k * B:(k + 1) * B],
                            x_sb[:, k * 128:(k + 1) * 128],
                            id16_sb[:],
                        )
                    xhi = sp.tile([128, KT * B], BF16, tag="x2")
                    nc.scalar.activation(xhi[:], ps_xt[:], ACTF.Relu)

                    # gates z = x @ Wih_c.T + const  [B,512], 2-term bf16
                    ps_z = psp.tile([B, GSL], F32, tag="psz")
                    for k in range(KT):
                        hi = slice(k * B, (k + 1) * B)
                        w = slice(k * GSL, (k + 1) * GSL)
                        nc.tensor.matmul(ps_z[:], xhi[:, hi], wihh_sb[:, w],
                                         start=(k == 0), stop=False)
                        nc.tensor.matmul(ps_z[:], xhi[:, hi], wihl_sb[:, w],
                                         start=False, stop=False)
                    nc.tensor.matmul(ps_z[:], id16_sb[:], const_sb[:],
                                     start=False, stop=True)
                    z_sb = ps_z

                    # cell (sigmoid via tanh):  h2 = (1+to)*tanh(c) = 2*h
                    # c = (1+tf)*(0.5*c0) + 0.5*(1+ti)*tg
                    tif = sp.tile([B, 256], F32, tag="tif")
                    tg = sp.tile([B, 128], F32, tag="tg")
                    to = sp.tile([B, 128], F32, tag="to")
                    nc.scalar.activation(tif[:], z_sb[:, 0:256], ACTF.Tanh, scale=0.5)
                    nc.scalar.activation(tg[:], z_sb[:, 256:384], ACTF.Tanh)
                    nc.scalar.activation(to[:], z_sb[:, 384:512], ACTF.Tanh, scale=0.5)
                    t1 = sp.tile([B, 128], F32, tag="t1")
                    t2 = sp.tile([B, 128], F32, tag="t2")
                    cc = sp.tile([B, 128], F32, tag="cc")
                    nc.vector.scalar_tensor_tensor(t1[:], tif[:, 128:256], 1.0,
                                                   c0h_sb[:], ALU.add, ALU.mult)
                    nc.vector.scalar_tensor_tensor(t2[:], tif[:, 0:128], 1.0,
                                                   tg[:], ALU.add, ALU.mult)
                    nc.vector.scalar_tensor_tensor(cc[:], t2[:], 0.5, t1[:],
                                                   ALU.mult, ALU.add)
                    tcc = sp.tile([B, 128], F32, tag="tcc")
                    nc.scalar.activation(tcc[:], cc[:], ACTF.Tanh)
                    h2 = sp.tile([B, 128], F32, tag="h2")
                    nc.vector.scalar_tensor_tensor(h2[:], to[:], 1.0, tcc[:],
                                                   ALU.add, ALU.mult)

                    # h^T chunk, bf16 split -> AllGather -> [128, KT*B] hi/lo
                    ps_h = psp.tile([128, B], F32, tag="psh")
                    nc.tensor.transpose(ps_h[:], h2[:], id16_sb[:])
                    hhi_c = sp.tile([128, B], BF16, tag="hhi_c")
                    hlo_c = sp.tile([128, B], BF16, tag="hlo_c")
                    nc.vector.tensor_copy(hhi_c[:], ps_h[:])
                    nc.vector.tensor_sub(hlo_c[:], ps_h[:], hhi_c[:])
                    cin1 = dp.tile([128, 2 * B], BF16, tag="cin1")
                    cout1 = dp.tile([128 * NC, 2 * B], BF16, tag="cout1")
                    nc.sync.dma_start(cin1[:, 0:B], hhi_c[:])
                    nc.sync.dma_start(cin1[:, B:2 * B], hlo_c[:])
                    if coll:
                        nc.gpsimd.collective_compute(
                            "AllGather", ALU.bypass,
                            replica_groups=[list(range(NC))],
                            ins=[cin1[:].opt()],
                            outs=[cout1[:].opt()],
                        )
                    else:
                        nc.sync.dma_start(cout1[0:128, :], cin1[:])
                    if warm:
                        for d in range(NWARM1):
                            nc.tensor.matmul(
                                ps_warm[:], id16_sb[:],
                                fcbr_sb[:, (d % NCHUNK) * CW:((d % NCHUNK) + 1) * CW],
                                start=(d == 0), stop=(d == NWARM1 - 1))
                    hT2v = hT2_sb[:].rearrange("p (k m) -> p k m", m=3 * B)
                    nc.sync.dma_start(
                        hT2v[:, :, 0:B],
                        cout1[:, 0:B].rearrange("(k p) b -> p k b", p=128),
                    )
                    nc.sync.dma_start(
                        hT2v[:, :, 2 * B:3 * B],
                        cout1[:, B:2 * B].rearrange("(k p) b -> p k b", p=128),
                    )

                    # fc: logits = 2h @ (0.5 fcW_c.T) + fc_b   [B, Vc]
                    logits_sb = wp.tile([B, Vc], F32, tag="logits")
                    vmax = sp.tile([B, NCHUNK * 8], F32, tag="vmax")
                    imax = sp.tile([B, NCHUNK * 8], U32, tag="imax")
                    for n in range(NCHUNK):
                        ps_f = psfc.tile([B, CW], F32, tag="psf")
                        cs = slice(n * CW, (n + 1) * CW)
                        for k in range(KT):
                            hi = slice(k * 3 * B, k * 3 * B + B)
                            lo = slice(k * 3 * B + 2 * B, (k + 1) * 3 * B)
                            w = slice(k * Vc + n * CW, k * Vc + (n + 1) * CW)
                            nc.tensor.matmul(ps_f[:], hT2_sb[:, hi], fcwh_sb[:, w],
                                             start=(k == 0), stop=False)
                            nc.tensor.matmul(ps_f[:], hT2_sb[:, hi], fcwl_sb[:, w],
                                             start=False, stop=False)
                            nc.tensor.matmul(ps_f[:], hT2_sb[:, lo], fcwh_sb[:, w],
                                             start=False, stop=(k == KT - 1))
                        nc.vector.scalar_tensor_tensor(
                            logits_sb[:, cs], ps_f[:], 1.0, fcbr_sb[:, cs],
                            ALU.mult, ALU.add)
                        nc.vector.max(vmax[:, n * 8:(n + 1) * 8], logits_sb[:, cs])
                        nc.vector.max_index(imax[:, n * 8:(n + 1) * 8],
                                            vmax[:, n * 8:(n + 1) * 8],
                                            logits_sb[:, cs])

                    # local candidate: (value, global vocab idx), first-max ties
                    cv = vmax[:].rearrange("p (n j) -> p n j", j=8)[:, :, 0]
                    ci = imax[:].rearrange("p (n j) -> p n j", j=8)[:, :, 0]
                    cif = sp.tile([B, NCHUNK], F32, tag="cif")
                    nc.vector.tensor_copy(cif[:], ci)
                    gidx = sp.tile([B, NCHUNK], F32, tag="gidx")
                    nc.vector.tensor_add(gidx[:], cif[:], cbase_sb[:])
                    pk = sp.tile([B, 2], F32, tag="pk")
                    lmax = pk[:, 0:1]
                    nc.vector.tensor_reduce(lmax, cv, axis=AX.X, op=ALU.max)
                    eq = sp.tile([B, NCHUNK], U8, tag="eq")
                    nc.vector.tensor_scalar(eq[:], cv, lmax, None, ALU.is_equal)
                    mi = sp.tile([B, NCHUNK], F32, tag="mi")
                    nc.vector.select(mi[:], eq[:], gidx[:], big8_sb[:])
                    nc.vector.tensor_reduce(pk[:, 1:2], mi[:], axis=AX.X,
                                            op=ALU.min)

                    # AllGather candidates [1,32] -> [8,32]
                    cin2 = dp.tile([1, 2 * B], F32, tag="cin2")
                    cout2 = dp.tile([NC, 2 * B], F32, tag="cout2")
                    nc.sync.dma_start(
                        cin2[0, :].rearrange("(j b) -> b j", b=B), pk[:])
                    if coll:
                        nc.gpsimd.collective_compute(
                            "AllGather", ALU.bypass,
                            replica_groups=[list(range(NC))],
                            ins=[cin2[:].opt()],
                            outs=[cout2[:].opt()],
                        )
                    else:
                        nc.sync.dma_start(cout2[0:1, :], cin2[:])
                    if warm:
                        for d in range(NWARM2):
                            nc.tensor.matmul(
                                ps_warm[:], id16_sb[:],
                                fcbr_sb[:, (d % NCHUNK) * CW:((d % NCHUNK) + 1) * CW],
                                start=(d == 0), stop=(d == NWARM2 - 1))
                    gv = sp.tile([B, NC], F32, tag="gv")
                    gi = sp.tile([B, NC], F32, tag="gi")
                    nc.sync.dma_start(gv[:], cout2[:, 0:B].rearrange("c b -> b c"))
                    nc.sync.dma_start(gi[:], cout2[:, B:2 * B].rearrange("c b -> b c"))

                    # global winner -> tok[t+1]; gmax for softmax
                    nc.vector.tensor_reduce(gmax_all[:, t:t + 1], gv[:], axis=AX.X,
                                            op=ALU.max)
                    eq2 = sp.tile([B, NC], U8, tag="eq2")
                    nc.vector.tensor_scalar(eq2[:], gv[:], gmax_all[:, t:t + 1], None,
                                            ALU.is_equal)
                    mi2 = sp.tile([B, NC], F32, tag="mi2")
                    nc.vector.select(mi2[:], eq2[:], gi[:], big8_sb[:])
                    wtok = sp.tile([B, 1], F32, tag="wtok")
                    nc.vector.tensor_reduce(wtok[:], mi2[:], axis=AX.X, op=ALU.min)
                    if not coll:
                        nc.vector.tensor_scalar(wtok[:], wtok[:], float(V - 1), 1.0,
                                                ALU.min, ALU.max)
                    nc.vector.tensor_copy(tok_all[:, t + 1:t + 2], wtok[:])

                    # store raw logits; fused exp+sum for logsumexp
                    nc.sync.dma_start(ldram[:, t * Vc:(t + 1) * Vc], logits_sb[:])
                    ngm = sp.tile([B, 1], F32, tag="ngm")
                    nc.vector.tensor_scalar_mul(ngm[:], gmax_all[:, t:t + 1], -1.0)
                    nc.scalar.activation(logits_sb[:], logits_sb[:], ACTF.Exp,
                                         bias=ngm[:], scale=1.0,
                                         accum_out=sume_all[:, t:t + 1])

                # ---------------- tail: log_softmax ----------------
                cinS = dp.tile([B, T], F32, tag="cinS")
                coutS = dp.tile([B * NC, T], F32, tag="coutS")
                nc.sync.dma_start(cinS[:], sume_all[:])
                if coll:
                    nc.gpsimd.collective_compute(
                        "AllGather", ALU.bypass,
                        replica_groups=[list(range(NC))],
                        ins=[cinS[:].opt()],
                        outs=[coutS[:].opt()],
                    )
                else:
                    nc.sync.dma_start(coutS[0:B, :], cinS[:])
                se_sb = pp.tile([B, T * NC], F32, tag="se")
                nc.sync.dma_start(
                    se_sb[:].rearrange("b (t c) -> b t c", c=NC),
                    coutS[:].rearrange("(c b) t -> b t c", b=B),
                )
                gsum = pp.tile([B, T], F32, tag="gsum")
                nc.vector.tensor_reduce(gsum[:],
                                        se_sb[:].rearrange("b (t c) -> b t c", c=NC),
                                        axis=AX.X, op=ALU.add)
                lns = pp.tile([B, T], F32, tag="lns")
                nc.scalar.activation(lns[:], gsum[:], ACTF.Ln)
                nshift = pp.tile([B, T], F32, tag="nshift")
                nc.vector.tensor_add(nshift[:], gmax_all[:], lns[:])
                nc.vector.tensor_scalar_mul(nshift[:], nshift[:], -1.0)

                for t in range(T):
                    fin = wp.tile([B, Vc], F32, tag="logits")
                    nc.sync.dma_start(fin[:], ldram[:, t * Vc:(t + 1) * Vc])
                    if t % 2 == 0:
                        nc.scalar.activation(fin[:], fin[:], ACTF.Identity,
                                             bias=nshift[:, t:t + 1])
                    else:
                        nc.vector.tensor_scalar_add(fin[:], fin[:],
                                                    nshift[:, t:t + 1])
                    nc.sync.dma_start(out_d[:, t, :], fin[:])

                nc.sync.dma_start(tokdbg_d[:, :], tok_all[:])
                if warm:
                    wsb = sp.tile([B, CW], F32, tag="x")
                    nc.scalar.copy(wsb[:], ps_warm[:])
                    nc.sync.dma_start(wdbg_d[:, :], wsb[:])

    nc.compile()
    return nc


def _bf16_split(a):
    hi = a.astype(ml_dtypes.bfloat16)
    lo = (a - hi.astype(np.float32)).astype(ml_dtypes.bfloat16)
    return hi, lo


def _prep_inputs(emb, h0, c0, W_ih, W_hh, b_ih, b_hh, fc_W, fc_b):
    emb = np.ascontiguousarray(np.asarray(emb, np.float32))
    h0 = np.asarray(h0, np.float32)
    c0 = np.asarray(c0, np.float32)
    W_ih = np.asarray(W_ih, np.float32)
    W_hh = np.asarray(W_hh, np.float32)
    bsum_full = (np.asarray(b_ih, np.float32) + np.asarray(b_hh, np.float32))
    fc_W = np.asarray(fc_W, np.float32)
    fc_b = np.asarray(fc_b, np.float32)
    id16 = np.eye(B, dtype=np.float32)
    h0t = np.ascontiguousarray(h0[0].T).reshape(KT, 128, B)

    in_maps = []
    for c in range(NC):
        rows = slice(c * Vc, (c + 1) * Vc)
        fcwt = np.ascontiguousarray(fc_W[rows].T * np.float32(0.5))
        fcwh, fcwl = _bf16_split(fcwt)
        gsl = [slice(g * H + c * 128, g * H + (c + 1) * 128) for g in range(4)]
        wih_c = np.concatenate([W_ih[s] for s in gsl], axis=0)       # [512, H]
        whh_c = np.concatenate([W_hh[s] for s in gsl], axis=0)
        wiht = np.ascontiguousarray(wih_c.T)
        wihh, wihl = _bf16_split(wiht)
        whht = np.ascontiguousarray(whh_c.T).reshape(KT, 128, GSL)
        bsum = np.concatenate([bsum_full[s] for s in gsl]).reshape(1, GSL)
        cbase = np.tile((c * Vc + CW * np.arange(NCHUNK, dtype=np.float32)), (B, 1))
        in_maps.append({
            "emb": emb,
            "fcwh": np.ascontiguousarray(fcwh.reshape(KT, 128, Vc)),
            "fcwl": np.ascontiguousarray(fcwl.reshape(KT, 128, Vc)),
            "wihh": np.ascontiguousarray(wihh.reshape(KT, 128, GSL)),
            "wihl": np.ascontiguousarray(wihl.reshape(KT, 128, GSL)),
            "whht": whht,
            "h0t": h0t,
            "c0h": np.ascontiguousarray(
                c0[0][:, c * 128:(c + 1) * 128] * np.float32(0.5)),
            "bsum": np.ascontiguousarray(bsum),
            "fcbr": np.ascontiguousarray(np.tile(fc_b[rows], (B, 1))),
            "id16": id16,
            "cbase": np.ascontiguousarray(cbase.astype(np.float32)),
        })
    return in_maps


def kernel(encoder_outputs=None, h0=None, c0=None, emb=None, W_ih=None, W_hh=None,
           b_ih=None, b_hh=None, fc_W=None, fc_b=None, **_unused):
    if "nc" not in _CACHE:
        _CACHE["nc"] = _build()
    nc = _CACHE["nc"]
    in_maps = _prep_inputs(emb, h0, c0, W_ih, W_hh, b_ih, b_hh, fc_W, fc_b)
    res = bass_utils.run_bass_kernel_spmd(nc, in_maps, list(range(NC)))
    out = np.concatenate([res.results[c]["out"] for c in range(NC)], axis=2)
    _CACHE["last_results"] = res
    return out



# revision 7
# speedup vs baseline: 1.2786x; 1.2786x over previous
"""Trainium2 Bass kernel for nn_Decoder: 11-step greedy LSTM decoder.

B=16, H=1024, V=32000, T=11 on 8 NeuronCores.
Sharding: tensor-parallel over vocab for the fc/logits matmul (each core holds
a [1024,4000] transposed shard of fc_W resident in SBUF), tensor-parallel over
hidden dim for the LSTM gates (each core computes h for its 128 hidden dims),
with a per-step AllGather of h^T chunks and an AllGather of per-core argmax
candidates (greedy feedback).  h0/c0 are never updated (reference semantics),
so h0@W_hh.T + b_ih + b_hh is precomputed on the host.

The fc matmul runs in 2 bf16 passes with a packed 48-column stationary
[hhi | 0 | hlo] (W = Whi+Wlo, h = hhi+hlo): pass1 streams Whi, pass2 streams
Wlo, producing all four cross terms in one PSUM tile (rows 0:16 and 32:48,
which the DVE then sums) — ~2^-16 relative precision at 2/3 the PE cost of a
3-term scheme.  The gates matmul uses a 2-term split (x single bf16,
W = Whi+Wlo): the dropped x-residual term contributes ~2e-5 logit error vs a
~6e-3 minimum top-2 gap, so the greedy path is unchanged.  The exact 2x from
the tanh-form sigmoid is folded into pre-scaled fc weights (0.5*fc_W).
fc weights are streamed chunk-major so step-0 fc overlaps the weight upload.
"""

import numpy as np
import ml_dtypes

import concourse.bass as bass
import concourse.bacc as bacc
import concourse.tile as tile
import concourse.mybir as mybir
from concourse import bass_utils

B = 16
H = 1024
V = 32000
T = 11
NC = 8
SOS = 1
Vc = V // NC          # 4000 vocab rows per core
KT = H // 128         # 8 contraction tiles
NCHUNK = 8            # fc free-dim chunks
CW = Vc // NCHUNK     # 500 cols per chunk
KW = KT * CW          # per-chunk sbuf weight stride
GSL = 4 * 128         # 512 gate rows per core
NWARM1 = 12           # PE-warming dummies during h-AllGather
NWARM2 = 14           # PE-warming dummies during candidate-AllGather

F32 = mybir.dt.float32
BF16 = mybir.dt.bfloat16
U32 = mybir.dt.uint32
U8 = mybir.dt.uint8
AX = mybir.AxisListType
ALU = mybir.AluOpType
ACTF = mybir.ActivationFunctionType
BIG = 1.0e9

_CACHE: dict = {}


def _build(reps=1, startup_in_rep=True, coll=True, warm=False):
    nc = bacc.Bacc("TRN2", target_bir_lowering=False, debug=False, num_devices=NC)

    emb_d = nc.dram_tensor("emb", [V, H], F32, kind="ExternalInput")
    fcwh_d = nc.dram_tensor("fcwh", [128, NCHUNK, KT * CW], BF16, kind="ExternalInput")
    fcwl_d = nc.dram_tensor("fcwl", [128, NCHUNK, KT * CW], BF16, kind="ExternalInput")
    wihh_d = nc.dram_tensor("wihh", [KT, 128, GSL], BF16, kind="ExternalInput")
    wihl_d = nc.dram_tensor("wihl", [KT, 128, GSL], BF16, kind="ExternalInput")
    constg_d = nc.dram_tensor("constg", [B, GSL], F32, kind="ExternalInput")
    c0h_d = nc.dram_tensor("c0h", [B, 128], F32, kind="ExternalInput")
    fcbr_d = nc.dram_tensor("fcbr", [B, Vc], F32, kind="ExternalInput")
    id16_d = nc.dram_tensor("id16", [B, B], F32, kind="ExternalInput")
    cbase_d = nc.dram_tensor("cbase", [B, NCHUNK], F32, kind="ExternalInput")

    out_d = nc.dram_tensor("out", [B, T, Vc], F32, kind="ExternalOutput")
    tokdbg_d = nc.dram_tensor("tokdbg", [B, T + 1], U32, kind="ExternalOutput")
    wdbg_d = nc.dram_tensor("wdbg", [B, CW], F32, kind="ExternalOutput")

    with tile.TileContext(nc) as tc:
        with (
            tc.tile_pool(name="persist", bufs=1) as pp,
            tc.tile_pool(name="work", bufs=2) as wp,
            tc.tile_pool(name="small", bufs=1) as sp,
            tc.tile_pool(name="psum", bufs=1, space="PSUM") as psp,
            tc.tile_pool(name="psfc", bufs=3, space="PSUM") as psfc,
            tc.tile_pool(name="dram", bufs=2, space="DRAM") as dp,
        ):
            # ---------------- persistent tiles ----------------
            fcwh_sb = pp.tile([128, NCHUNK * KW], BF16, tag="fcwh")
            fcwl_sb = pp.tile([128, NCHUNK * KW], BF16, tag="fcwl")
            wihh_sb = pp.tile([128, KT * GSL], BF16, tag="wihh")
            wihl_sb = pp.tile([128, KT * GSL], BF16, tag="wihl")
            fcbr_sb = pp.tile([B, Vc], F32, tag="fcbr")
            const_sb = pp.tile([B, GSL], F32, tag="const")
            c0h_sb = pp.tile([B, 128], F32, tag="c0h")
            id16_sb = pp.tile([B, B], F32, tag="id16")
            cbase_sb = pp.tile([B, NCHUNK], F32, tag="cbase")
            big8_sb = pp.tile([B, NCHUNK], F32, tag="big8")
            hT2_sb = pp.tile([128, KT * 3 * B], BF16, tag="hT2")
            gmax_all = pp.tile([B, T], F32, tag="gmax")
            sume_all = pp.tile([B, T], F32, tag="sume")
            tok_all = pp.tile([B, T + 1], U32, tag="tok")
            ldram = dp.tile([B, T * Vc], F32, tag="ldram")

            ps_warm = psp.tile([B, CW], F32, tag="warm") if warm else None

            for _rep in range(reps):
                if _rep == 0 or startup_in_rep:
                    # ---------------- startup ----------------
                    nc.sync.dma_start(const_sb[:], constg_d[:, :])
                    nc.sync.dma_start(c0h_sb[:], c0h_d[:, :])
                    nc.sync.dma_start(id16_sb[:], id16_d[:, :])
                    nc.sync.dma_start(cbase_sb[:], cbase_d[:, :])
                    for k in range(KT):
                        nc.sync.dma_start(wihh_sb[:, k * GSL:(k + 1) * GSL],
                                          wihh_d[k, :, :])
                        nc.sync.dma_start(wihl_sb[:, k * GSL:(k + 1) * GSL],
                                          wihl_d[k, :, :])
                    # chunk-major fc weights: chunk n arrives before fc uses it
                    for n in range(NCHUNK):
                        nc.sync.dma_start(fcwh_sb[:, n * KW:(n + 1) * KW],
                                          fcwh_d[:, n, :])
                        nc.sync.dma_start(fcwl_sb[:, n * KW:(n + 1) * KW],
                                          fcwl_d[:, n, :])
                    nc.sync.dma_start(fcbr_sb[:], fcbr_d[:, :])
                    nc.vector.memset(big8_sb[:], BIG)
                    nc.vector.memset(hT2_sb[:], 0)

                nc.vector.memset(tok_all[:], 0)
                nc.vector.memset(tok_all[:, 0:1], SOS)

                # ---------------- decode loop ----------------
                for t in range(T):
                    # gather x = emb[tok]  -> [B, H]
                    x_sb = sp.tile([B, H], F32, tag="x")
                    nc.gpsimd.indirect_dma_start(
                        out=x_sb[:],
                        out_offset=None,
                        in_=emb_d[:, :],
                        in_offset=bass.IndirectOffsetOnAxis(
                            ap=tok_all[:, t:t + 1], axis=0),
                    )

                    # transpose x -> [128, KT*B], relu, bf16
                    ps_xt = psp.tile([128, KT * B], F32, tag="psxt")
                    for k in range(KT):
                        nc.tensor.transpose(
                            ps_xt[:, k * B:(k + 1) * B],
                            x_sb[:, k * 128:(k + 1) * 128],
                            id16_sb[:],
                        )
                    xhi = sp.tile([128, KT * B], BF16, tag="x2")
                    nc.scalar.activation(xhi[:], ps_xt[:], ACTF.Relu)

                    # gates z = x @ Wih_c.T + const  [B,512], 2-term bf16
                    ps_z = psp.tile([B, GSL], F32, tag="psz")
                    for k in range(KT):
                        hi = slice(k * B, (k + 1) * B)
                        w = slice(k * GSL, (k + 1) * GSL)
                        nc.tensor.matmul(ps_z[:], xhi[:, hi], wihh_sb[:, w],
                                         start=(k == 0), stop=False)
                        nc.tensor.matmul(ps_z[:], xhi[:, hi], wihl_sb[:, w],
                                         start=False, stop=False)
                    nc.tensor.matmul(ps_z[:], id16_sb[:], const_sb[:],
                                     start=False, stop=True)
                    z_sb = ps_z

                    # cell (sigmoid via tanh):  h2 = (1+to)*tanh(c) = 2*h
                    # c = (1+tf)*(0.5*c0) + 0.5*(1+ti)*tg
                    tif = sp.tile([B, 256], F32, tag="tif")
                    tg = sp.tile([B, 128], F32, tag="tg")
                    to = sp.tile([B, 128], F32, tag="to")
                    nc.scalar.activation(tif[:], z_sb[:, 0:256], ACTF.Tanh, scale=0.5)
                    nc.scalar.activation(tg[:], z_sb[:, 256:384], ACTF.Tanh)
                    nc.scalar.activation(to[:], z_sb[:, 384:512], ACTF.Tanh, scale=0.5)
                    t1 = sp.tile([B, 128], F32, tag="t1")
                    t2 = sp.tile([B, 128], F32, tag="t2")
                    cc = sp.tile([B, 128], F32, tag="cc")
                    nc.vector.scalar_tensor_tensor(t1[:], tif[:, 128:256], 1.0,
                                                   c0h_sb[:], ALU.add, ALU.mult)
                    nc.vector.scalar_tensor_tensor(t2[:], tif[:, 0:128], 1.0,
                                                   tg[:], ALU.add, ALU.mult)
                    nc.vector.scalar_tensor_tensor(cc[:], t2[:], 0.5, t1[:],
                                                   ALU.mult, ALU.add)
                    tcc = sp.tile([B, 128], F32, tag="tcc")
                    nc.scalar.activation(tcc[:], cc[:], ACTF.Tanh)
                    h2 = sp.tile([B, 128], F32, tag="h2")
                    nc.vector.scalar_tensor_tensor(h2[:], to[:], 1.0, tcc[:],
                                                   ALU.add, ALU.mult)

                    # h^T chunk, bf16 split -> AllGather -> hT2 [hi | 0 | lo]
                    ps_h = psp.tile([128, B], F32, tag="psh")
                    nc.tensor.transpose(ps_h[:], h2[:], id16_sb[:])
                    hsp = sp.tile([128, 2 * B], BF16, tag="hsp")
                    nc.vector.tensor_copy(hsp[:, 0:B], ps_h[:])
                    nc.vector.tensor_sub(hsp[:, B:2 * B], ps_h[:], hsp[:, 0:B])
                    cin1 = dp.tile([128, 2 * B], BF16, tag="cin1")
                    cout1 = dp.tile([128 * NC, 2 * B], BF16, tag="cout1")
                    nc.sync.dma_start(cin1[:], hsp[:])
                    if coll:
                        nc.gpsimd.collective_compute(
                            "AllGather", ALU.bypass,
                            replica_groups=[list(range(NC))],
                            ins=[cin1[:].opt()],
                            outs=[cout1[:].opt()],
                        )
                    else:
                        nc.sync.dma_start(cout1[0:128, :], cin1[:])
                    if warm:
                        for d in range(NWARM1):
                            nc.tensor.matmul(
                                ps_warm[:], id16_sb[:],
                                fcbr_sb[:, (d % NCHUNK) * CW:((d % NCHUNK) + 1) * CW],
                                start=(d == 0), stop=(d == NWARM1 - 1))
                    hT2v = hT2_sb[:].rearrange("p (k m) -> p k m", m=3 * B)
                    nc.sync.dma_start(
                        hT2v[:, :, 0:B],
                        cout1[:, 0:B].rearrange("(k p) b -> p k b", p=128),
                    )
                    nc.sync.dma_start(
                        hT2v[:, :, 2 * B:3 * B],
                        cout1[:, B:2 * B].rearrange("(k p) b -> p k b", p=128),
                    )

                    # fc: logits = 2h @ (0.5 fcW_c.T) + fc_b   [B, Vc]
                    # packed stationary [hhi | 0 | hlo] -> psum rows 0:16 & 32:48
                    logits_sb = wp.tile([B, Vc], F32, tag="logits")
                    vmax = sp.tile([B, NCHUNK * 8], F32, tag="vmax")
                    imax = sp.tile([B, NCHUNK * 8], U32, tag="imax")
                    for n in range(NCHUNK):
                        ps_f = psfc.tile([3 * B, CW], F32, tag="psf")
                        cs = slice(n * CW, (n + 1) * CW)
                        for k in range(KT):
                            st = slice(k * 3 * B, (k + 1) * 3 * B)
                            w = slice(n * KW + k * CW, n * KW + (k + 1) * CW)
                            nc.tensor.matmul(ps_f[:], hT2_sb[:, st], fcwh_sb[:, w],
                                             start=(k == 0), stop=False)
                            nc.tensor.matmul(ps_f[:], hT2_sb[:, st], fcwl_sb[:, w],
                                             start=False, stop=(k == KT - 1))
                        tmp_sb = wp.tile([B, CW], F32, tag="fctmp")
                        nc.vector.scalar_tensor_tensor(
                            tmp_sb[:], ps_f[2 * B:3 * B, :], 1.0, fcbr_sb[:, cs],
                            ALU.mult, ALU.add)
                        nc.vector.scalar_tensor_tensor(
                            logits_sb[:, cs], ps_f[0:B, :], 1.0, tmp_sb[:],
                            ALU.mult, ALU.add)
                        nc.vector.max(vmax[:, n * 8:(n + 1) * 8], logits_sb[:, cs])
                        nc.vector.max_index(imax[:, n * 8:(n + 1) * 8],
                                            vmax[:, n * 8:(n + 1) * 8],
                                            logits_sb[:, cs])

                    # local candidate: (value, global vocab idx), first-max ties
                    cv = vmax[:].rearrange("p (n j) -> p n j", j=8)[:, :, 0]
                    ci = imax[:].rearrange("p (n j) -> p n j", j=8)[:, :, 0]
                    cif = sp.tile([B, NCHUNK], F32, tag="cif")
                    nc.vector.tensor_copy(cif[:], ci)
                    gidx = sp.tile([B, NCHUNK], F32, tag="gidx")
                    nc.vector.tensor_add(gidx[:], cif[:], cbase_sb[:])
                    pk = sp.tile([B, 2], F32, tag="pk")
                    lmax = pk[:, 0:1]
                    nc.vector.tensor_reduce(lmax, cv, axis=AX.X, op=ALU.max)
                    eq = sp.tile([B, NCHUNK], U8, tag="eq")
                    nc.vector.tensor_scalar(eq[:], cv, lmax, None, ALU.is_equal)
                    mi = sp.tile([B, NCHUNK], F32, tag="mi")
                    nc.vector.select(mi[:], eq[:], gidx[:], big8_sb[:])
                    nc.vector.tensor_reduce(pk[:, 1:2], mi[:], axis=AX.X,
                                            op=ALU.min)

                    # AllGather candidates [1,32] -> [8,32]
                    cin2 = dp.tile([1, 2 * B], F32, tag="cin2")
                    cout2 = dp.tile([NC, 2 * B], F32, tag="cout2")
                    nc.sync.dma_start(
                        cin2[0, :].rearrange("(j b) -> b j", b=B), pk[:])
                    if coll:
                        nc.gpsimd.collective_compute(
                            "AllGather", ALU.bypass,
                            replica_groups=[list(range(NC))],
                            ins=[cin2[:].opt()],
                            outs=[cout2[:].opt()],
                        )
                    else:
                        nc.sync.dma_start(cout2[0:1, :], cin2[:])
                    if warm:
                        for d in range(NWARM2):
                            nc.tensor.matmul(
                                ps_warm[:], id16_sb[:],
                                fcbr_sb[:, (d % NCHUNK) * CW:((d % NCHUNK) + 1) * CW],
                                start=(d == 0), stop=(d == NWARM2 - 1))
                    gv = sp.tile([B, NC], F32, tag="gv")
                    gi = sp.tile([B, NC], F32, tag="gi")
                    nc.sync.dma_start(gv[:], cout2[:, 0:B].rearrange("c b -> b c"))
                    nc.sync.dma_start(gi[:], cout2[:, B:2 * B].rearrange("c b -> b c"))

                    # global winner -> tok[t+1]; gmax for softmax
                    nc.vector.tensor_reduce(gmax_all[:, t:t + 1], gv[:], axis=AX.X,
                                            op=ALU.max)
                    eq2 = sp.tile([B, NC], U8, tag="eq2")
                    nc.vector.tensor_scalar(eq2[:], gv[:], gmax_all[:, t:t + 1], None,
                                            ALU.is_equal)
                    mi2 = sp.tile([B, NC], F32, tag="mi2")
                    nc.vector.select(mi2[:], eq2[:], gi[:], big8_sb[:])
                    wtok = sp.tile([B, 1], F32, tag="wtok")
                    nc.vector.tensor_reduce(wtok[:], mi2[:], axis=AX.X, op=ALU.min)
                    if not coll:
                        nc.vector.tensor_scalar(wtok[:], wtok[:], float(V - 1), 1.0,
                                                ALU.min, ALU.max)
                    nc.vector.tensor_copy(tok_all[:, t + 1:t + 2], wtok[:])

                    # store raw logits; fused exp+sum for logsumexp
                    nc.sync.dma_start(ldram[:, t * Vc:(t + 1) * Vc], logits_sb[:])
                    ngm = sp.tile([B, 1], F32, tag="ngm")
                    nc.vector.tensor_scalar_mul(ngm[:], gmax_all[:, t:t + 1], -1.0)
                    nc.scalar.activation(logits_sb[:], logits_sb[:], ACTF.Exp,
                                         bias=ngm[:], scale=1.0,
                                         accum_out=sume_all[:, t:t + 1])

                # ---------------- tail: log_softmax ----------------
                cinS = dp.tile([B, T], F32, tag="cinS")
                coutS = dp.tile([B * NC, T], F32, tag="coutS")
                nc.sync.dma_start(cinS[:], sume_all[:])
                if coll:
                    nc.gpsimd.collective_compute(
                        "AllGather", ALU.bypass,
                        replica_groups=[list(range(NC))],
                        ins=[cinS[:].opt()],
                        outs=[coutS[:].opt()],
                    )
                else:
                    nc.sync.dma_start(coutS[0:B, :], cinS[:])
                se_sb = pp.tile([B, T * NC], F32, tag="se")
                nc.sync.dma_start(
                    se_sb[:].rearrange("b (t c) -> b t c", c=NC),
                    coutS[:].rearrange("(c b) t -> b t c", b=B),
                )
                gsum = pp.tile([B, T], F32, tag="gsum")
                nc.vector.tensor_reduce(gsum[:],
                                        se_sb[:].rearrange("b (t c) -> b t c", c=NC),
                                        axis=AX.X, op=ALU.add)
                lns = pp.tile([B, T], F32, tag="lns")
                nc.scalar.activation(lns[:], gsum[:], ACTF.Ln)
                nshift = pp.tile([B, T], F32, tag="nshift")
                nc.vector.tensor_add(nshift[:], gmax_all[:], lns[:])
                nc.vector.tensor_scalar_mul(nshift[:], nshift[:], -1.0)

                for t in range(T):
                    fin = wp.tile([B, Vc], F32, tag="logits")
                    nc.sync.dma_start(fin[:], ldram[:, t * Vc:(t + 1) * Vc])
                    if t % 2 == 0:
                        nc.scalar.activation(fin[:], fin[:], ACTF.Identity,
                                             bias=nshift[:, t:t + 1])
                    else:
                        nc.vector.tensor_scalar_add(fin[:], fin[:],
                                                    nshift[:, t:t + 1])
                    nc.sync.dma_start(out_d[:, t, :], fin[:])

                nc.sync.dma_start(tokdbg_d[:, :], tok_all[:])
                if warm:
                    wsb = sp.tile([B, CW], F32, tag="x")
                    nc.scalar.copy(wsb[:], ps_warm[:])
                    nc.sync.dma_start(wdbg_d[:, :], wsb[:])

    nc.compile()
    return nc


def _bf16_split(a):
    hi = a.astype(ml_dtypes.bfloat16)
    lo = (a - hi.astype(np.float32)).astype(ml_dtypes.bfloat16)
    return hi, lo


def _prep_inputs(emb, h0, c0, W_ih, W_hh, b_ih, b_hh, fc_W, fc_b):
    emb = np.ascontiguousarray(np.asarray(emb, np.float32))
    h0 = np.asarray(h0, np.float32)
    c0 = np.asarray(c0, np.float32)
    W_ih = np.asarray(W_ih, np.float32)
    W_hh = np.asarray(W_hh, np.float32)
    bsum_full = (np.asarray(b_ih, np.float32) + np.asarray(b_hh, np.float32))
    fc_W = np.asarray(fc_W, np.float32)
    fc_b = np.asarray(fc_b, np.float32)
    id16 = np.eye(B, dtype=np.float32)
    h0b = h0[0]  # [B, H]

    in_maps = []
    for c in range(NC):
        rows = slice(c * Vc, (c + 1) * Vc)
        fcwt = np.ascontiguousarray(fc_W[rows].T * np.float32(0.5))
        fcwh, fcwl = _bf16_split(fcwt)
        # [H, Vc] -> partition-first chunk-major [128, NCHUNK, KT*CW]
        fcwh = fcwh.reshape(KT, 128, NCHUNK, CW).transpose(
            1, 2, 0, 3).reshape(128, NCHUNK, KT * CW)
        fcwl = fcwl.reshape(KT, 128, NCHUNK, CW).transpose(
            1, 2, 0, 3).reshape(128, NCHUNK, KT * CW)
        gsl = [slice(g * H + c * 128, g * H + (c + 1) * 128) for g in range(4)]
        wih_c = np.concatenate([W_ih[s] for s in gsl], axis=0)       # [512, H]
        whh_c = np.concatenate([W_hh[s] for s in gsl], axis=0)
        wiht = np.ascontiguousarray(wih_c.T)
        wihh, wihl = _bf16_split(wiht)
        bsum = np.concatenate([bsum_full[s] for s in gsl])           # [512]
        constg = h0b @ whh_c.T + bsum[None, :]                       # [B, 512]
        cbase = np.tile((c * Vc + CW * np.arange(NCHUNK, dtype=np.float32)), (B, 1))
        in_maps.append({
            "emb": emb,
            "fcwh": np.ascontiguousarray(fcwh),
            "fcwl": np.ascontiguousarray(fcwl),
            "wihh": np.ascontiguousarray(wihh.reshape(KT, 128, GSL)),
            "wihl": np.ascontiguousarray(wihl.reshape(KT, 128, GSL)),
            "constg": np.ascontiguousarray(constg.astype(np.float32)),
            "c0h": np.ascontiguousarray(
                c0[0][:, c * 128:(c + 1) * 128] * np.float32(0.5)),
            "fcbr": np.ascontiguousarray(np.tile(fc_b[rows], (B, 1))),
            "id16": id16,
            "cbase": np.ascontiguousarray(cbase.astype(np.float32)),
        })
    return in_maps


def kernel(encoder_outputs=None, h0=None, c0=None, emb=None, W_ih=None, W_hh=None,
           b_ih=None, b_hh=None, fc_W=None, fc_b=None, **_unused):
    if "nc" not in _CACHE:
        _CACHE["nc"] = _build()
    nc = _CACHE["nc"]
    in_maps = _prep_inputs(emb, h0, c0, W_ih, W_hh, b_ih, b_hh, fc_W, fc_b)
    res = bass_utils.run_bass_kernel_spmd(nc, in_maps, list(range(NC)))
    out = np.concatenate([res.results[c]["out"] for c in range(NC)], axis=2)
    _CACHE["last_results"] = res
    return out
